# revision 2
# baseline (speedup 1.0000x reference)
"""Trainium2 Bass kernel for nn_DistanceLoss (retrieval_knn, 5-way few-shot
temporal-tuple distance logits).

Math (per the reference):
  tuples = C(8,3) = 56 frame triples; embed dim 1024; supports grouped 5/class.
  qe = relu(q_tuples @ W.T + b); se likewise.
  logits[q,c] = -mean_t min_s ||qe[q,t] - se[c,s]||

Key restructuring on device:
  1. The 6144-wide gather-matmul is factorized: P[j,(a,f)] per frame (7x fewer
     FLOPs), then tuple-combine via run-grouped adds.  Frame columns are
     f-major and tuple columns t-major so every combine op streams contiguous
     50-element (query) / 25-element (support) inner runs.
  2. Both large matmul stages (frame projection and q.s dots) run in
     fp8e4m3 with perf_mode=DoubleRow (two K-chunks per pass, ~1.5x PE
     throughput).  W is pre-scaled x16 on host so its entries sit in the
     fp8 normal range; the 1/16 is folded into the PSUM drain.
  3. dist^2 = q2 + (s2 - 2*dot): s2 is broadcast to all partitions via an
     all-ones matmul, the class-min is scalar_tensor_tensor + reduce-min on
     DVE, and q2 is added as the per-partition bias of the ACT relu.
  4. mean over the 56 tuples is a [128,5].T @ [128,50] block-ones matmul.
  5. Squares for q2/s2 are elementwise tensor_mul on DVE (off the ACT
     engine), computed from the same fp8 values the dots matmul consumes,
     so dist^2 stays consistent (non-negative).

Sharding: data-parallel over queries (50/core on 8 cores); support set and
weights replicated; host concatenates the per-core [5,50] outputs.
"""
import sys

sys.path.insert(0, '/opt/trn_rl_repo')
import numpy as np
import ml_dtypes
from itertools import combinations
from contextlib import ExitStack

from concourse import bass, bacc, tile, mybir
from concourse.bass_utils import run_bass_kernel_spmd

BF16 = ml_dtypes.bfloat16
FP8 = ml_dtypes.float8_e4m3
F32 = mybir.dt.float32
BF = mybir.dt.bfloat16
F8 = mybir.dt.float8e4
RELU = mybir.ActivationFunctionType.Relu
SQRT = mybir.ActivationFunctionType.Sqrt
DROW = mybir.MatmulPerfMode.DoubleRow

WAY, TSS = 5, 3
NS, NQ, SEQ, D = 25, 400, 8, 2048
D2 = 1024
NCORES = 8
QPC = NQ // NCORES              # 50 queries per core
SHOT = NS // WAY                # 5
TUP = list(combinations(range(SEQ), TSS))
TN = len(TUP)                   # 56
QT = QPC * TN                   # 2800 query-tuple columns per core
NMT = (QT + 127) // 128         # 22 M-tiles
QTP = NMT * 128                 # 2816 (padded)
STT = NS * TN                   # 1400 support-tuple columns
STTP = 1408                     # padded to %16 for DoubleRow jc-pair stride
SPC = STT // WAY                # 280 per class
KC = D // 128                   # 16 contraction chunks for P
KCP = KC // 2                   # 8 DoubleRow k-chunk pairs
JCN = D2 // 128                 # 8 embed-dim chunks
JCP = JCN // 2                  # 4 DoubleRow jc pairs for dots
FQ = QPC * SEQ                  # 400 query frame-columns
FS = NS * SEQ                   # 200 support frame-columns
F = FQ + FS                     # 600
F2 = 608                        # padded to %16 for DoubleRow kc-pair stride
WG = TSS * 128                  # 384 W columns per (kc, jc) group
WSCALE = 16.0                   # host pre-scale on W so fp8 sees normals
NRUN = None

# Tuple order is OURS to choose (the class-min and tuple-mean are symmetric
# in tuple order).  Sorting tuples by (f1, f0, f2) makes every group of runs
# with equal f1 a single affine access pattern: for fixed f1, f0 spans
# 0..f1-1 and f2 spans f1+1..7, so one op covers f1*(7-f1) tuples.
# Group (f1): nf0 = f1 runs, each of length ln = 7-f1.
# rbase = number of runs before the group; tbase = number of tuples before.
F1G = []
_rb, _tb = 0, 0
for _f1 in range(1, 7):
    _ln = 7 - _f1
    F1G.append((_f1, _ln, _rb, _tb))
    _rb += _f1
    _tb += _f1 * _ln
NRUN = _rb                      # 21
assert _tb == TN


def build_nc():
    nc = bacc.Bacc("TRN2", target_bir_lowering=False, debug=False)
    # qt: [d, f*50+q], st: [d, f*25+s] (s class-sorted), w: [d, jc*384+a*128+jj]
    qt_d = nc.dram_tensor("qt", [D, FQ], F8, kind="ExternalInput")
    st_d = nc.dram_tensor("st", [D, FS], F8, kind="ExternalInput")
    w_d = nc.dram_tensor("w", [D, TSS * D2], F8, kind="ExternalInput")
    b_d = nc.dram_tensor("b", [128, JCN], F32, kind="ExternalInput")
    bm_d = nc.dram_tensor("bm", [128, NMT * QPC], BF, kind="ExternalInput")
    out_d = nc.dram_tensor("out", [WAY, QPC], F32, kind="ExternalOutput")

    with tile.TileContext(nc) as tc, ExitStack() as ctx:
        ep = ctx.enter_context
        wt_pool = ep(tc.tile_pool(name="wt", bufs=3))
        qst_pool = ep(tc.tile_pool(name="qst", bufs=1))
        dr_pool = ep(tc.tile_pool(name="dr", bufs=9))
        s01_pool = ep(tc.tile_pool(name="s01", bufs=2))
        tmp_pool = ep(tc.tile_pool(name="tmp", bufs=2))
        qe_pool = ep(tc.tile_pool(name="qe", bufs=1))
        se_pool = ep(tc.tile_pool(name="se", bufs=1))
        sq_pool = ep(tc.tile_pool(name="sq", bufs=2))
        row_pool = ep(tc.tile_pool(name="row", bufs=1))
        misc_pool = ep(tc.tile_pool(name="misc", bufs=1))
        eps_pool = ep(tc.tile_pool(name="eps", bufs=3))

        # constants / persistent rows
        ones = misc_pool.tile([128, 1], BF, tag="ones")
        nc.vector.memset(ones[:], 1.0)
        onesq = misc_pool.tile([128, 128], BF, tag="onesq")
        nc.vector.memset(onesq[:], 1.0)
        b_sb = misc_pool.tile([128, JCN], F32, tag="bsb")
        nc.sync.dma_start(b_sb[:], b_d.ap())
        bm_sb = misc_pool.tile([128, NMT * QPC], BF, tag="bm")
        s2nb = row_pool.tile([128, STT], F32, tag="s2nb")
        q2c = row_pool.tile([128, NMT], F32, tag="q2c")
        sq_total = row_pool.tile([128, QTP], BF, tag="sq_total")
        sqs_total = row_pool.tile([128, STT], BF, tag="sqs_total")

        # frames: queries (cols 0:FQ) and supports (cols FQ:F), K on
        # partitions; cols F:F2 are zero pad so the DoubleRow pair stride
        # (F2) is a multiple of 16.
        qstt = qst_pool.tile([128, KC * F2], F8, tag="qst")
        qst_v = qstt.rearrange('p (kc f) -> p kc f', f=F2)
        nc.vector.memset(qst_v[:, :, F:F2], 0.0)
        wt0 = wt_pool.tile([128, KC * WG], F8, tag="wt", name="wt0")
        wt0_v = wt0.rearrange('p (kc c) -> p kc c', c=WG)
        w_v = w_d.ap().rearrange('(kc p) c -> p kc c', p=128)
        st_v = st_d.ap().rearrange('(kc p) c -> p kc c', p=128)
        qt_v = qt_d.ap().rearrange('(kc p) c -> p kc c', p=128)
        for g in range(0, KC, 4):
            nc.sync.dma_start(wt0_v[:, g:g + 4], w_v[:, g:g + 4, 0:WG])
            nc.sync.dma_start(qst_v[:, g:g + 4, FQ:F], st_v[:, g:g + 4])
        for g in range(0, KC, 4):
            nc.sync.dma_start(qst_v[:, g:g + 4, 0:FQ], qt_v[:, g:g + 4])
        nc.sync.dma_start(bm_sb[:], bm_d.ap())

        # qe/se as single tiles so the dots matmul can pair jc chunks with a
        # constant stride (QTP / STTP, both %16 == 0).
        qe_big = qe_pool.tile([128, JCN * QTP], F8, tag="qe")
        se_big = se_pool.tile([128, JCN * STTP], F8, tag="se")

        with tc.tile_pool(name="pp", bufs=4, space="PSUM") as pp_pool, \
             tc.tile_pool(name="ps", bufs=4, space="PSUM") as ps_pool:

            for jc in range(JCN):
                # ---- P matmuls for this embed chunk (fp8 DoubleRow) ----
                if jc == 0:
                    wtj = wt0
                    wtj_v = wt0_v
                else:
                    wtj = wt_pool.tile([128, KC * WG], F8, tag="wt",
                                       name=f"wt{jc}")
                    wtj_v = wtj.rearrange('p (kc c) -> p kc c', c=WG)
                    for g in range(0, KC, 4):
                        nc.sync.dma_start(
                            wtj_v[:, g:g + 4],
                            w_v[:, g:g + 4, jc * WG:(jc + 1) * WG])
                psq, pss = [], []
                for a in range(TSS):
                    pq = pp_pool.tile([128, FQ], F32, tag="pp",
                                      name=f"pq_{jc}_{a}")
                    pg = ps_pool.tile([128, FS], F32, tag="ps",
                                      name=f"pg_{jc}_{a}")
                    for g in range(KCP):
                        lhs = wtj_v[:, 2 * g:2 * g + 2, a * 128:(a + 1) * 128]
                        nc.tensor.matmul(pq[:], lhs,
                                         qst_v[:, 2 * g:2 * g + 2, 0:FQ],
                                         start=(g == 0), stop=(g == KCP - 1),
                                         perf_mode=DROW)
                        nc.tensor.matmul(pg[:], lhs,
                                         qst_v[:, 2 * g:2 * g + 2, FQ:F],
                                         start=(g == 0), stop=(g == KCP - 1),
                                         perf_mode=DROW)
                    psq.append(pq)
                    pss.append(pg)
                bcol = b_sb[:, jc:jc + 1]
                # drain all three a-chunks to bf16 SBUF; the 1/WSCALE undo of
                # the host-side W prescale is folded in, and the bias lands
                # at a=0 (tensor_scalar two-op: (psum * 1/16) + b).
                dr = []
                for a in range(TSS):
                    d = dr_pool.tile([128, F], BF, tag="dr", name=f"dr_{jc}_{a}")
                    if a == 0:
                        nc.vector.tensor_scalar(
                            d[:, 0:FQ], psq[0][:], 1.0 / WSCALE, bcol,
                            mybir.AluOpType.mult, mybir.AluOpType.add)
                        nc.vector.tensor_scalar(
                            d[:, FQ:F], pss[0][:], 1.0 / WSCALE, bcol,
                            mybir.AluOpType.mult, mybir.AluOpType.add)
                    else:
                        nc.vector.tensor_scalar_mul(d[:, 0:FQ], psq[a][:],
                                                    1.0 / WSCALE)
                        nc.vector.tensor_scalar_mul(d[:, FQ:F], pss[a][:],
                                                    1.0 / WSCALE)
                    dr.append(d)
                d0, d1, d2 = dr

                # ---- tuple combine: queries (DVE), 6+6 dense grouped ops ----
                # S01[(rb+f0)*50+q] = P0[f0*50+q] + P1[f1*50+q] for f0 < f1
                s01q = s01_pool.tile([128, NRUN * QPC], BF, tag="s01q")
                for (f1, ln, rb, tb) in F1G:
                    nc.vector.tensor_add(
                        s01q[:, rb * QPC:(rb + f1) * QPC]
                        .rearrange('p (f q) -> p f q', q=QPC),
                        d0[:, 0:f1 * QPC]
                        .rearrange('p (f q) -> p f q', q=QPC),
                        d1[:, f1 * QPC:(f1 + 1) * QPC]
                        .unsqueeze(1).broadcast_to((128, f1, QPC)))
                tmpq = tmp_pool.tile([128, QT], BF, tag="tmpq")
                for (f1, ln, rb, tb) in F1G:
                    o = tmpq[:, tb * QPC:(tb + f1 * ln) * QPC] \
                        .rearrange('p (f t q) -> p f t q', t=ln, q=QPC)
                    i0 = s01q[:, rb * QPC:(rb + f1) * QPC] \
                        .rearrange('p (f q) -> p f q', q=QPC) \
                        .unsqueeze(2).broadcast_to((128, f1, ln, QPC))
                    i1 = d2[:, (f1 + 1) * QPC:SEQ * QPC] \
                        .rearrange('p (t q) -> p t q', q=QPC) \
                        .unsqueeze(1).broadcast_to((128, f1, ln, QPC))
                    nc.vector.tensor_add(o, i0, i1)
                qe = qe_big[:, jc * QTP:(jc + 1) * QTP]
                nc.vector.memset(qe[:, QT:QTP], 0.0)
                nc.scalar.activation(qe[:, 0:QT], tmpq[:], RELU)

                # ---- tuple combine: supports (GPSIMD), same grouping ----
                s01s = s01_pool.tile([128, NRUN * NS], BF, tag="s01s")
                for (f1, ln, rb, tb) in F1G:
                    nc.gpsimd.tensor_add(
                        s01s[:, rb * NS:(rb + f1) * NS]
                        .rearrange('p (f q) -> p f q', q=NS),
                        d0[:, FQ:FQ + f1 * NS]
                        .rearrange('p (f q) -> p f q', q=NS),
                        d1[:, FQ + f1 * NS:FQ + (f1 + 1) * NS]
                        .unsqueeze(1).broadcast_to((128, f1, NS)))
                tmps = tmp_pool.tile([128, STT], BF, tag="tmps")
                for (f1, ln, rb, tb) in F1G:
                    o = tmps[:, tb * NS:(tb + f1 * ln) * NS] \
                        .rearrange('p (f t q) -> p f t q', t=ln, q=NS)
                    i0 = s01s[:, rb * NS:(rb + f1) * NS] \
                        .rearrange('p (f q) -> p f q', q=NS) \
                        .unsqueeze(2).broadcast_to((128, f1, ln, NS))
                    i1 = d2[:, FQ + (f1 + 1) * NS:FQ + SEQ * NS] \
                        .rearrange('p (t q) -> p t q', q=NS) \
                        .unsqueeze(1).broadcast_to((128, f1, ln, NS))
                    nc.gpsimd.tensor_add(o, i0, i1)
                sett = tmp_pool.tile([128, STT], F8, tag="sett")
                nc.scalar.activation(sett[:], tmps[:], RELU)
                # reorder t-major -> class-major: out[c*280+sh*56+t] = in[t*25+c*5+sh]
                se = se_big[:, jc * STTP:jc * STTP + STT]
                se_v = se.rearrange('p (c sh t) -> p c sh t', sh=SHOT, t=TN)
                in_v = sett.rearrange('p (t c sh) -> p c sh t', c=WAY, sh=SHOT)
                nc.vector.tensor_copy(se_v, in_v)

                # ---- squares (DVE mul of the fp8 values) accumulated in bf16
                if jc == 0:
                    nc.vector.tensor_mul(sq_total[:], qe[:], qe[:])
                    nc.vector.tensor_mul(sqs_total[:], se[:], se[:])
                else:
                    sq = sq_pool.tile([128, QTP], BF, tag="sq")
                    nc.vector.tensor_mul(sq[:], qe[:], qe[:])
                    nc.vector.tensor_add(sq_total[:], sq_total[:], sq[:])
                    sqs = sq_pool.tile([128, STT], BF, tag="sqs")
                    nc.vector.tensor_mul(sqs[:], se[:], se[:])
                    nc.vector.tensor_add(sqs_total[:], sqs_total[:], sqs[:])

        qe3 = qe_big.rearrange('p (jc c) -> p jc c', c=QTP)
        se3 = se_big.rearrange('p (jc c) -> p jc c', c=STTP)

        # ---- dots + class-min + sqrt + tuple-mean ----
        with tc.tile_pool(name="pd", bufs=7, space="PSUM") as pd_pool, \
             tc.tile_pool(name="aux", bufs=1, space="PSUM") as aux_pool, \
             tc.tile_pool(name="dsb", bufs=NMT) as dsb_pool, \
             tc.tile_pool(name="scr", bufs=3) as scr_pool:

            def emit_norm_tail():
                # q2 columns per M-tile: out[p,0] = sum_p' sq_total[p', mt*128+p]
                # s2 broadcast to all partitions in one shot: all-ones [128,128]
                # stationary x sqs_total chunk -> every partition = column sum.
                # Emitted after mt=0's dots matmuls: nothing on PE waits on it.
                for mt in range(NMT):
                    op = aux_pool.tile([128, 512], F32, tag="aux", name=f"oq_{mt}")
                    nc.tensor.matmul(op[:, 0:1],
                                     sq_total[:, mt * 128:(mt + 1) * 128],
                                     ones[:], start=True, stop=True)
                    nc.vector.tensor_copy(q2c[:, mt:mt + 1], op[:, 0:1])
                for lo in range(0, STT, 512):
                    hi = min(lo + 512, STT)
                    op = aux_pool.tile([128, 512], F32, tag="aux",
                                      name=f"ob_{lo}")
                    nc.tensor.matmul(op[:, 0:hi - lo], onesq[:],
                                     sqs_total[:, lo:hi], start=True, stop=True)
                    nc.vector.tensor_copy(s2nb[:, lo:hi], op[:, 0:hi - lo])

            dsb_tiles = []
            for mt in range(NMT):
                pds = [pd_pool.tile([128, SPC], F32, tag="pd",
                                    name=f"pd_{mt}_{c}") for c in range(WAY)]
                for g in range(JCP):
                    lhs = qe3[:, 2 * g:2 * g + 2, mt * 128:(mt + 1) * 128]
                    for c in range(WAY):
                        nc.tensor.matmul(
                            pds[c][:], lhs,
                            se3[:, 2 * g:2 * g + 2, c * SPC:(c + 1) * SPC],
                            start=(g == 0), stop=(g == JCP - 1),
                            perf_mode=DROW)
                if mt == 0:
                    emit_norm_tail()
                mred = eps_pool.tile([128, WAY], F32, tag="mred")
                scr = scr_pool.tile([128, WAY * SPC], BF, tag="scr",
                                    name=f"scr_{mt}")
                for c in range(WAY):
                    # scr = s2 - 2*dot
                    nc.vector.scalar_tensor_tensor(
                        out=scr[:, c * SPC:(c + 1) * SPC],
                        in0=pds[c][:], scalar=-2.0,
                        in1=s2nb[:, c * SPC:(c + 1) * SPC],
                        op0=mybir.AluOpType.mult, op1=mybir.AluOpType.add)
                # mred[:, c] = min_s(scr[:, c, :]) in one fused reduce
                nc.vector.tensor_reduce(
                    mred[:], scr.rearrange('p (c s) -> p c s', s=SPC),
                    axis=mybir.AxisListType.X, op=mybir.AluOpType.min)
                r1 = eps_pool.tile([128, WAY], F32, tag="r1")
                nc.scalar.activation(r1[:], mred[:], RELU,
                                     bias=q2c[:, mt:mt + 1])
                dsb = dsb_pool.tile([128, WAY], BF, tag="dsb",
                                    name=f"dsb_{mt}")
                nc.scalar.activation(dsb[:], r1[:], SQRT)
                dsb_tiles.append(dsb)
            # tuple-mean at the end so the PE stream never waits on ACT
            lpsum = aux_pool.tile([WAY, QPC], F32, tag="aux", name="lpsum")
            for mt in range(NMT):
                nc.tensor.matmul(lpsum[:], dsb_tiles[mt][:],
                                 bm_sb[:, mt * QPC:(mt + 1) * QPC],
                                 start=(mt == 0), stop=(mt == NMT - 1))
            outsb = misc_pool.tile([WAY, QPC], F32, tag="outsb")
            nc.scalar.mul(outsb[:], lpsum[:], -1.0 / TN)
            nc.sync.dma_start(out_d.ap(), outsb[:])

    nc.compile()
    return nc


_NC = None


def _get_nc():
    global _NC
    if _NC is None:
        _NC = build_nc()
    return _NC


def _host_prep(support_set, support_labels, queries, W, b):
    support_set = np.asarray(support_set)
    support_labels = np.asarray(support_labels)
    queries = np.asarray(queries)
    W = np.asarray(W)
    b = np.asarray(b)

    order = np.argsort(support_labels, kind='stable')
    S = support_set[order]                                        # class-major
    # st[d, f*25+s]
    st = np.ascontiguousarray(
        S.transpose(2, 1, 0).reshape(D, FS)).astype(FP8)
    # W[j, a*2048+d] -> w2[d, jc*384 + a*128 + jj],  j = jc*128+jj
    w2 = np.ascontiguousarray(
        (W * WSCALE).reshape(JCN, 128, TSS, D)
        .transpose(3, 0, 2, 1).reshape(D, TSS * D2)).astype(FP8)
    bsb = np.ascontiguousarray(b.reshape(JCN, 128).T).astype(np.float32)
    # qt' = t*50 + q  ->  q = qt' % 50
    bm = np.zeros((128, NMT * QPC), np.float32)
    for g in range(QT):
        mt, p = divmod(g, 128)
        bm[p, mt * QPC + g % QPC] = 1.0
    bmh = bm.astype(BF16)
    in_maps = []
    for c in range(NCORES):
        qs = queries[c * QPC:(c + 1) * QPC]
        # qt[d, f*50+q]
        qtc = np.ascontiguousarray(
            qs.transpose(2, 1, 0).reshape(D, FQ)).astype(FP8)
        in_maps.append({"qt": qtc, "st": st, "w": w2, "b": bsb, "bm": bmh})
    return in_maps


def kernel(support_set, support_labels, queries, W, b):
    in_maps = _host_prep(support_set, support_labels, queries, W, b)
    nc = _get_nc()
    res = run_bass_kernel_spmd(nc, in_maps, core_ids=list(range(NCORES)))
    outs = [np.asarray(res.results[c]["out"]).T for c in range(NCORES)]
    return np.ascontiguousarray(np.concatenate(outs, axis=0)).astype(np.float32)


# revision 10
# speedup vs baseline: 1.1305x; 1.1305x over previous
"""Trainium2 Bass kernel for nn_DistanceLoss (retrieval_knn, 5-way few-shot
temporal-tuple distance logits).

Math (per the reference):
  tuples = C(8,3) = 56 frame triples; embed dim 1024; supports grouped 5/class.
  qe = relu(q_tuples @ W.T + b); se likewise.
  logits[q,c] = -mean_t min_s ||qe[q,t] - se[c,s]||

Key restructuring on device:
  1. The 6144-wide gather-matmul is factorized: P[j,(a,f)] per frame (7x fewer
     FLOPs), then tuple-combine via run-grouped adds on DVE/GPSIMD reading
     the P chunks straight out of PSUM (no drain for a=0/1; a=2 is cast once).
  2. Both large matmul stages (frame projection and q.s dots) run in fp8e4m3
     with perf_mode=DoubleRow (two 128-deep K-chunks per pass).  W is
     pre-scaled x16 on host; the 1/16 and the bias fold into the ACT relu
     (out = relu(psum_sum/16 + b)), which also writes fp8 directly.
  3. Norms fold into the dots matmul as an extra K=2 bf16 chunk:
     psum = dot - q2/2 - s2/2 = -dist^2/2, so the class-min is a plain
     reduce-max over each class PSUM block, then dist = sqrt(-2*min(max,0)).
     q2/s2 come from ones-matmuls over fp8 squares of qe/se.
  4. mean over the 56 tuples is a [128,5].T @ [128,50] block-ones matmul.

Sharding: data-parallel over queries (50/core on 8 cores); support set and
weights replicated; host concatenates the per-core [5,50] outputs.
"""
import sys

sys.path.insert(0, '/opt/trn_rl_repo')
import numpy as np
import ml_dtypes
from itertools import combinations
from contextlib import ExitStack

from concourse import bass, bacc, tile, mybir
from concourse.bass_utils import run_bass_kernel_spmd

BF16 = ml_dtypes.bfloat16
FP8 = ml_dtypes.float8_e4m3
F32 = mybir.dt.float32
BF = mybir.dt.bfloat16
F8 = mybir.dt.float8e4
RELU = mybir.ActivationFunctionType.Relu
SQRT = mybir.ActivationFunctionType.Sqrt
SQUARE = mybir.ActivationFunctionType.Square
DROW = mybir.MatmulPerfMode.DoubleRow

WAY, TSS = 5, 3
NS, NQ, SEQ, D = 25, 400, 8, 2048
D2 = 1024
NCORES = 8
QPC = NQ // NCORES              # 50 queries per core
SHOT = NS // WAY                # 5
TUP = list(combinations(range(SEQ), TSS))
TN = len(TUP)                   # 56
QT = QPC * TN                   # 2800 query-tuple columns per core
NMT = (QT + 127) // 128         # 22 M-tiles
QTP = NMT * 128                 # 2816 (padded, %16 == 0)
STT = NS * TN                   # 1400 support-tuple columns
STTP = 1408                     # padded so the jc-pair stride is %16
SPC = STT // WAY                # 280 per class
KC = D // 128                   # 16 contraction chunks for P
KCP = KC // 2                   # 8 DoubleRow k-chunk pairs
JCN = D2 // 128                 # 8 embed-dim chunks
JCP = JCN // 2                  # 4 DoubleRow jc pairs for dots
FQ = QPC * SEQ                  # 400 query frame-columns
FS = NS * SEQ                   # 200 support frame-columns
F = FQ + FS                     # 600
F2 = 608                        # padded so the kc-pair stride is %16
WG = TSS * 128                  # 384 W columns per (kc, jc) group
WSCALE = 16.0                   # host pre-scale on W: fp8 normal range
NRUN = None

# Tuple order is OURS to choose (the class-min and tuple-mean are symmetric
# in tuple order).  Sorting tuples by (f1, f0, f2) makes every group of runs
# with equal f1 a single affine access pattern: for fixed f1, f0 spans
# 0..f1-1 and f2 spans f1+1..7, so one op covers f1*(7-f1) tuples.
F1G = []
_rb, _tb = 0, 0
for _f1 in range(1, 7):
    _ln = 7 - _f1
    F1G.append((_f1, _ln, _rb, _tb))
    _rb += _f1
    _tb += _f1 * _ln
NRUN = _rb                      # 21
assert _tb == TN

# engine split for the stage-2 query combine (by f1 group)
TMPQ_DVE = {2, 3, 4, 5}
TMPQ_GPS = {1, 6}


def build_nc():
    nc = bacc.Bacc("TRN2", target_bir_lowering=False, debug=False)
    # qt: [d, f*50+q], st: [d, f*25+s] (s class-sorted), w: [d, jc*384+a*128+jj]
    qt_d = nc.dram_tensor("qt", [D, FQ], F8, kind="ExternalInput")
    st_d = nc.dram_tensor("st", [D, FS], F8, kind="ExternalInput")
    w_d = nc.dram_tensor("w", [D, TSS * D2], F8, kind="ExternalInput")
    b_d = nc.dram_tensor("b", [128, JCN], F32, kind="ExternalInput")
    bm_d = nc.dram_tensor("bm", [128, NMT * QPC], BF, kind="ExternalInput")
    out_d = nc.dram_tensor("out", [WAY, QPC], F32, kind="ExternalOutput")

    with tile.TileContext(nc) as tc, ExitStack() as ctx:
        ep = ctx.enter_context
        wt_pool = ep(tc.tile_pool(name="wt", bufs=3))
        qst_pool = ep(tc.tile_pool(name="qst", bufs=1))
        d2_pool = ep(tc.tile_pool(name="d2", bufs=8))
        s01_pool = ep(tc.tile_pool(name="s01", bufs=2))
        tmp_pool = ep(tc.tile_pool(name="tmp", bufs=3))
        big_pool = ep(tc.tile_pool(name="big", bufs=1))
        row_pool = ep(tc.tile_pool(name="row", bufs=1))
        misc_pool = ep(tc.tile_pool(name="misc", bufs=1))
        eps_pool = ep(tc.tile_pool(name="eps", bufs=3))

        # constants
        ones2 = misc_pool.tile([128, 32], F8, tag="ones2")
        nc.vector.memset(ones2[:], 1.0)
        # lhsT [128, 2, 1] with pair step 16 (DoubleRow needs step%16==0)
        ones_pair = ones2[:].rearrange('p (k m) -> p k m', m=16)[:, :, 0:1]
        b_sb = misc_pool.tile([128, JCN], F32, tag="bsb")
        nc.sync.dma_start(b_sb[:], b_d.ap())
        bm_sb = misc_pool.tile([128, NMT * QPC], BF, tag="bm")

        # frames: queries (cols 0:FQ) and supports (cols FQ:F); cols F:F2 are
        # never streamed (pad only exists so the pair stride F2 is %16).
        qstt = qst_pool.tile([128, KC * F2], F8, tag="qst")
        qst_v = qstt.rearrange('p (kc f) -> p kc f', f=F2)
        wt0 = wt_pool.tile([128, KC * WG], F8, tag="wt", name="wt0")
        wt0_v = wt0.rearrange('p (kc c) -> p kc c', c=WG)
        w_v = w_d.ap().rearrange('(kc p) c -> p kc c', p=128)
        st_v = st_d.ap().rearrange('(kc p) c -> p kc c', p=128)
        qt_v = qt_d.ap().rearrange('(kc p) c -> p kc c', p=128)
        for g in range(0, KC, 4):
            nc.sync.dma_start(wt0_v[:, g:g + 4], w_v[:, g:g + 4, 0:WG])
            nc.sync.dma_start(qst_v[:, g:g + 4, FQ:F], st_v[:, g:g + 4])
        for g in range(0, KC, 4):
            nc.sync.dma_start(qst_v[:, g:g + 4, 0:FQ], qt_v[:, g:g + 4])
        nc.sync.dma_start(bm_sb[:], bm_d.ap())

        # single big tiles so DoubleRow jc-pairs have constant %16 strides
        qe_big = big_pool.tile([128, JCN * QTP], F8, tag="qe")
        se_big = big_pool.tile([128, JCN * STTP], F8, tag="se")
        sq_big = big_pool.tile([128, JCN * QTP], F8, tag="sq")
        sqs_big = big_pool.tile([128, JCN * STTP], F8, tag="sqs")

        with tc.tile_pool(name="pp", bufs=4, space="PSUM") as pp_pool, \
             tc.tile_pool(name="ps", bufs=4, space="PSUM") as ps_pool:

            for jc in range(JCN):
                # ---- P matmuls for this embed chunk (fp8 DoubleRow) ----
                if jc == 0:
                    wtj_v = wt0_v
                else:
                    wtj = wt_pool.tile([128, KC * WG], F8, tag="wt",
                                       name=f"wt{jc}")
                    wtj_v = wtj.rearrange('p (kc c) -> p kc c', c=WG)
                    for g in range(0, KC, 4):
                        nc.sync.dma_start(
                            wtj_v[:, g:g + 4],
                            w_v[:, g:g + 4, jc * WG:(jc + 1) * WG])
                psq, pss = [], []
                for a in range(TSS):
                    pq = pp_pool.tile([128, FQ], F32, tag="pp",
                                      name=f"pq_{jc}_{a}")
                    pg = ps_pool.tile([128, FS], F32, tag="ps",
                                      name=f"pg_{jc}_{a}")
                    for g in range(KCP):
                        lhs = wtj_v[:, 2 * g:2 * g + 2, a * 128:(a + 1) * 128]
                        nc.tensor.matmul(pq[:], lhs,
                                         qst_v[:, 2 * g:2 * g + 2, 0:FQ],
                                         start=(g == 0), stop=(g == KCP - 1),
                                         perf_mode=DROW)
                        nc.tensor.matmul(pg[:], lhs,
                                         qst_v[:, 2 * g:2 * g + 2, FQ:F],
                                         start=(g == 0), stop=(g == KCP - 1),
                                         perf_mode=DROW)
                    psq.append(pq)
                    pss.append(pg)
                bcol = b_sb[:, jc:jc + 1]

                # drains: plain dtype casts (scale/bias fold into the relu).
                # DVE and GPSIMD cannot read two PSUM operands (single PSUM
                # read port / no PSUM port), so combines run from SBUF bf16.
                # q-side casts on DVE, s-side casts on ACT (sits near PSUM).
                dq, ds = [], []
                for a in range(TSS):
                    dqa = d2_pool.tile([128, FQ], BF, tag="dq", name=f"dq{a}")
                    nc.vector.tensor_copy(dqa[:], psq[a][:])
                    dq.append(dqa)
                    dsa = d2_pool.tile([128, FS], BF, tag="ds", name=f"ds{a}")
                    nc.scalar.copy(dsa[:], pss[a][:])
                    ds.append(dsa)

                # ---- tuple combine: queries, stage 1 (DVE) ----
                # S01[(rb+f0)*50+q] = P0[f0*50+q] + P1[f1*50+q] for f0 < f1
                s01q = s01_pool.tile([128, NRUN * QPC], BF, tag="s01q")
                for (f1, ln, rb, tb) in F1G:
                    nc.vector.tensor_add(
                        s01q[:, rb * QPC:(rb + f1) * QPC]
                        .rearrange('p (f q) -> p f q', q=QPC),
                        dq[0][:, 0:f1 * QPC]
                        .rearrange('p (f q) -> p f q', q=QPC),
                        dq[1][:, f1 * QPC:(f1 + 1) * QPC]
                        .unsqueeze(1).broadcast_to((128, f1, QPC)))
                tmpq = tmp_pool.tile([128, QT], BF, tag="tmpq")
                for (f1, ln, rb, tb) in F1G:
                    o = tmpq[:, tb * QPC:(tb + f1 * ln) * QPC] \
                        .rearrange('p (f t q) -> p f t q', t=ln, q=QPC)
                    i0 = s01q[:, rb * QPC:(rb + f1) * QPC] \
                        .rearrange('p (f q) -> p f q', q=QPC) \
                        .unsqueeze(2).broadcast_to((128, f1, ln, QPC))
                    i1 = dq[2][:, (f1 + 1) * QPC:SEQ * QPC] \
                        .rearrange('p (t q) -> p t q', q=QPC) \
                        .unsqueeze(1).broadcast_to((128, f1, ln, QPC))
                    nc.vector.tensor_add(o, i0, i1)
                qe = qe_big[:, jc * QTP:(jc + 1) * QTP]
                nc.vector.memset(qe[:, QT:QTP], 0.0)
                # relu((P0+P1+P2)/16 + b) -> fp8
                nc.scalar.activation(qe[:, 0:QT], tmpq[:], RELU,
                                     bias=bcol, scale=1.0 / WSCALE)

                # ---- tuple combine: supports (GPSIMD), same grouping ----
                s01s = s01_pool.tile([128, NRUN * NS], BF, tag="s01s")
                for (f1, ln, rb, tb) in F1G:
                    nc.gpsimd.tensor_add(
                        s01s[:, rb * NS:(rb + f1) * NS]
                        .rearrange('p (f q) -> p f q', q=NS),
                        ds[0][:, 0:f1 * NS]
                        .rearrange('p (f q) -> p f q', q=NS),
                        ds[1][:, f1 * NS:(f1 + 1) * NS]
                        .unsqueeze(1).broadcast_to((128, f1, NS)))
                tmps = tmp_pool.tile([128, STT], BF, tag="tmps")
                for (f1, ln, rb, tb) in F1G:
                    o = tmps[:, tb * NS:(tb + f1 * ln) * NS] \
                        .rearrange('p (f t q) -> p f t q', t=ln, q=NS)
                    i0 = s01s[:, rb * NS:(rb + f1) * NS] \
                        .rearrange('p (f q) -> p f q', q=NS) \
                        .unsqueeze(2).broadcast_to((128, f1, ln, NS))
                    i1 = ds[2][:, (f1 + 1) * NS:SEQ * NS] \
                        .rearrange('p (t q) -> p t q', q=NS) \
                        .unsqueeze(1).broadcast_to((128, f1, ln, NS))
                    nc.gpsimd.tensor_add(o, i0, i1)
                # relu with the t-major -> class-major reorder folded in:
                # strided read of tmps, contiguous fp8 write of se.
                se = se_big[:, jc * STTP:jc * STTP + STT]
                se_v = se.rearrange('p (c sh t) -> p c sh t', sh=SHOT, t=TN)
                in_v = tmps.rearrange('p (t c sh) -> p c sh t', c=WAY, sh=SHOT)
                nc.scalar.activation(se_v, in_v, RELU,
                                     bias=bcol, scale=1.0 / WSCALE)

                # ---- squares (fp8, same values the dots matmul sees) ----
                sq = sq_big[:, jc * QTP:(jc + 1) * QTP]
                nc.vector.tensor_mul(sq[:], qe[:], qe[:])
                sqs = sqs_big[:, jc * STTP:jc * STTP + STT]
                nc.scalar.activation(sqs[:], se[:], SQUARE)

        qe3 = qe_big.rearrange('p (jc c) -> p jc c', c=QTP)
        se3 = se_big.rearrange('p (jc c) -> p jc c', c=STTP)
        sq3 = sq_big.rearrange('p (jc c) -> p jc c', c=QTP)
        sqs3 = sqs_big.rearrange('p (jc c) -> p jc c', c=STTP)

        # ---- dots + class-min + sqrt + tuple-mean ----
        with tc.tile_pool(name="pd", bufs=7, space="PSUM") as pd_pool, \
             tc.tile_pool(name="aux", bufs=1, space="PSUM") as aux_pool, \
             tc.tile_pool(name="dsb", bufs=NMT) as dsb_pool:

            # norm rows: nrm = [q2; 1] (bf16 lhsT), s2n2 = [-1/2; -s2/2]
            # (bf16 rhs); one K=2 matmul folds -q2/2 - s2/2 into each dots
            # PSUM block so it holds -dist^2/2 directly.
            # engines can only write starting at partition 0, so the 2-row
            # tiles are memset whole (constant row), the computed row lands
            # at partition 0 (nrm) or goes via a partition-0 scratch row +
            # SBUF->SBUF DMA into partition 1 (s2n2).
            nrm = row_pool.tile([2, QTP], BF, tag="nrm")
            nc.vector.memset(nrm[:], 1.0)
            s2n2 = row_pool.tile([2, STT], BF, tag="s2n2")
            nc.vector.memset(s2n2[:], -0.5)
            s2mrow = row_pool.tile([1, STT], BF, tag="s2mrow")
            for lo in range(0, QTP, 512):
                hi = min(lo + 512, QTP)
                op = aux_pool.tile([1, 512], F32, tag="aux", name=f"oq_{lo}")
                for g in range(JCP):
                    nc.tensor.matmul(op[0:1, 0:hi - lo], ones_pair,
                                     sq3[:, 2 * g:2 * g + 2, lo:hi],
                                     start=(g == 0), stop=(g == JCP - 1),
                                     perf_mode=DROW)
                nc.vector.tensor_copy(nrm[0:1, lo:hi], op[0:1, 0:hi - lo])
            for lo in range(0, STT, 512):
                hi = min(lo + 512, STT)
                op = aux_pool.tile([1, 512], F32, tag="aux", name=f"os_{lo}")
                for g in range(JCP):
                    nc.tensor.matmul(op[0:1, 0:hi - lo], ones_pair,
                                     sqs3[:, 2 * g:2 * g + 2, lo:hi],
                                     start=(g == 0), stop=(g == JCP - 1),
                                     perf_mode=DROW)
                nc.vector.tensor_scalar_mul(s2mrow[0:1, lo:hi],
                                            op[0:1, 0:hi - lo], -0.5)
            nc.sync.dma_start(s2n2[1:2, :], s2mrow[0:1, :])

            dsb_tiles = []
            for mt in range(NMT):
                pds = [pd_pool.tile([128, SPC], F32, tag="pd",
                                    name=f"pd_{mt}_{c}") for c in range(WAY)]
                for g in range(JCP):
                    lhs = qe3[:, 2 * g:2 * g + 2, mt * 128:(mt + 1) * 128]
                    for c in range(WAY):
                        nc.tensor.matmul(
                            pds[c][:], lhs,
                            se3[:, 2 * g:2 * g + 2, c * SPC:(c + 1) * SPC],
                            start=(g == 0), stop=False, perf_mode=DROW)
                nlhs = nrm[0:2, mt * 128:(mt + 1) * 128]
                for c in range(WAY):
                    nc.tensor.matmul(pds[c][:], nlhs,
                                     s2n2[0:2, c * SPC:(c + 1) * SPC],
                                     start=False, stop=True)
                # mred[:, c] = max_s(-dist^2/2)  (free-axis reduce: DVE only)
                mred = eps_pool.tile([128, WAY], F32, tag="mred")
                for c in range(WAY):
                    eng = nc.vector
                    eng.tensor_reduce(
                        mred[:, c:c + 1], pds[c][:],
                        axis=mybir.AxisListType.X, op=mybir.AluOpType.max)
                mredc = eps_pool.tile([128, WAY], F32, tag="mredc")
                nc.vector.tensor_scalar_min(mredc[:], mred[:], 0.0)
                dsb = dsb_pool.tile([128, WAY], BF, tag="dsb",
                                    name=f"dsb_{mt}")
                nc.scalar.activation(dsb[:], mredc[:], SQRT, scale=-2.0)
                dsb_tiles.append(dsb)
            # tuple-mean at the end so the PE stream never waits on ACT
            lpsum = aux_pool.tile([WAY, QPC], F32, tag="aux", name="lpsum")
            for mt in range(NMT):
                nc.tensor.matmul(lpsum[:], dsb_tiles[mt][:],
                                 bm_sb[:, mt * QPC:(mt + 1) * QPC],
                                 start=(mt == 0), stop=(mt == NMT - 1))
            outsb = misc_pool.tile([WAY, QPC], F32, tag="outsb")
            nc.scalar.mul(outsb[:], lpsum[:], -1.0 / TN)
            nc.sync.dma_start(out_d.ap(), outsb[:])

    nc.compile()
    return nc


_NC = None


def _get_nc():
    global _NC
    if _NC is None:
        _NC = build_nc()
    return _NC


def _host_prep(support_set, support_labels, queries, W, b):
    support_set = np.asarray(support_set)
    support_labels = np.asarray(support_labels)
    queries = np.asarray(queries)
    W = np.asarray(W)
    b = np.asarray(b)

    order = np.argsort(support_labels, kind='stable')
    S = support_set[order]                                        # class-major
    # st[d, f*25+s]
    st = np.ascontiguousarray(
        S.transpose(2, 1, 0).reshape(D, FS)).astype(FP8)
    # W[j, a*2048+d] -> w2[d, jc*384 + a*128 + jj],  j = jc*128+jj
    w2 = np.ascontiguousarray(
        (W * WSCALE).reshape(JCN, 128, TSS, D)
        .transpose(3, 0, 2, 1).reshape(D, TSS * D2)).astype(FP8)
    bsb = np.ascontiguousarray(b.reshape(JCN, 128).T).astype(np.float32)
    # qt' = t*50 + q  ->  q = qt' % 50
    bm = np.zeros((128, NMT * QPC), np.float32)
    for g in range(QT):
        mt, p = divmod(g, 128)
        bm[p, mt * QPC + g % QPC] = 1.0
    bmh = bm.astype(BF16)
    in_maps = []
    for c in range(NCORES):
        qs = queries[c * QPC:(c + 1) * QPC]
        # qt[d, f*50+q]
        qtc = np.ascontiguousarray(
            qs.transpose(2, 1, 0).reshape(D, FQ)).astype(FP8)
        in_maps.append({"qt": qtc, "st": st, "w": w2, "b": bsb, "bm": bmh})
    return in_maps


def kernel(support_set, support_labels, queries, W, b):
    in_maps = _host_prep(support_set, support_labels, queries, W, b)
    nc = _get_nc()
    res = run_bass_kernel_spmd(nc, in_maps, core_ids=list(range(NCORES)))
    outs = [np.asarray(res.results[c]["out"]).T for c in range(NCORES)]
    return np.ascontiguousarray(np.concatenate(outs, axis=0)).astype(np.float32)
